# revision 6
# baseline (speedup 1.0000x reference)
"""Trainium2 Bass kernel for nn_CustomModel_52338471469275 (dense MLP).

Computes out = relu(input @ (S*THETA)^T + bias) @ weight + bias2
  input  [2048, 8192] f32
  S,THETA[1024, 8192] f32   (fused on host into W1 = S*THETA)
  weight [1024, 1024] f32
  out    [2048, 1024] f32

Sharding over 8 NeuronCores: 4 batch groups (512 rows each) x 2 hidden
halves (512 of the 1024 hidden units each).  Core (i, j) computes

  fT_ij  = relu(W1[jblk] @ x[iblk]^T + bias[jblk])          # [512, 512]
  outT_p = weight[jblk]^T @ fT_ij                           # [1024, 512]

i.e. a partial (contraction-split) second GEMM.  The host sums the two
j-partials per batch group, transposes, and adds bias2.  No on-device
collectives needed.

GEMM1 runs on the PE in fp8 (e4m3) DoubleRow mode at 2x the bf16 rate,
using a split-precision scheme that keeps ~bf16 accuracy:
  x  = x_hi + x_lo      (both e4m3; x_lo is the quantization residual)
  w' = 256*(S*THETA) = w_hi + w_lo   (scaled so w' ~ N(0,4) stays clear
                                      of the e4m3 subnormal floor)
  256*logits = x_hi.w_hi + x_lo.w_hi + x_hi.w_lo      (x_lo.w_lo dropped)
Three chains accumulate into the same fp32 PSUM bank; the 2^-8 descale
is folded into the host-side w2 (= weight/256), so f_sb holds 256*f and
no extra on-device ops are needed.  GEMM2 stays bf16.  Measured
end-to-end relative error vs the fp32 reference is ~2e-3 (absmax),
slightly better than the all-bf16 variant.
"""

import os
import sys

import numpy as np

if "/opt/trn_rl_repo" not in sys.path:
    sys.path.insert(0, "/opt/trn_rl_repo")

import ml_dtypes

import concourse.bass as bass
import concourse.tile as tile
from concourse import mybir
from concourse._compat import checkenv
from concourse.bass_utils import run_bass_kernel_spmd

# The image's antenv stub lacks axon_hooks; if BASS_TRACE is set in the
# environment, run_bass_kernel_spmd imports it unconditionally. Provide a
# no-op fallback (trace is skipped, compile+run still work) unless a real
# hook module is already installed.
try:
    import antenv.axon_hooks  # noqa: F401
except ImportError:
    import types

    import antenv

    _hooks = types.ModuleType("antenv.axon_hooks")
    _hooks._hook = None
    _hooks.set_axon_ntff_profile_hook = lambda h: setattr(_hooks, "_hook", h)
    _hooks.get_axon_ntff_profile_hook = lambda: _hooks._hook
    sys.modules["antenv.axon_hooks"] = _hooks
    antenv.axon_hooks = _hooks

B, O, I = 2048, 1024, 8192
R, C = 4, 2                 # batch groups x hidden halves
BS, OS = B // R, O // C     # 512, 512
P = 128
N = BS                      # moving free dim per matmul
KT1 = I // P                # 64 k-tiles, GEMM1
DKT = KT1 // 2              # 32 double-k-tiles (DoubleRow does 256 deep)
MT1 = OS // P               # 4 m-tiles, GEMM1
KT2 = OS // P               # 4 k-tiles, GEMM2
MT2 = O // P                # 8 m-tiles, GEMM2
SC = 256.0                  # fp8 weight pre-scale (descale folded into w2)

BF16 = mybir.dt.bfloat16
F32 = mybir.dt.float32
FP8 = mybir.dt.float8e4
DR = mybir.MatmulPerfMode.DoubleRow

# k-tiles per slab DMA for GEMM1 (even sizes only: DoubleRow pairs
# consecutive k-tiles within a block; small blocks first so the PE gets
# data early)
SCHED = [2, 2, 4] + [8] * 7
assert sum(SCHED) == KT1 and all(q % 2 == 0 for q in SCHED)
QMAX = max(SCHED)


def _blockize(aT):
    """Rewrite [8192, W] so each SCHED block of QK k-tiles is stored p-major
    ([P, QK, W] C-order): one contiguous QK*W-element descriptor per SBUF
    partition instead of QK separate rows."""
    out = np.empty_like(aT)
    kt0 = 0
    for QK in SCHED:
        blk = aT[kt0 * P : (kt0 + QK) * P]
        out[kt0 * P : (kt0 + QK) * P] = (
            blk.reshape(QK, P, -1).transpose(1, 0, 2).reshape(QK * P, -1)
        )
        kt0 += QK
    return out

_CACHE = {}
LAST_RESULTS = None  # BassKernelResults of the most recent run (for test.py)


def _split_multi_waits(nc, max_waits=1):
    """This container's walrus codegen rejects instructions carrying more
    than one semaphore wait ("Too many sync wait commands", CoreV3GenImpl).
    Tile's kernel-tail drain aggregates several; hoist the extras onto
    preceding same-engine NoOps (identical semantics: engines execute their
    stream in order)."""
    for fn in nc.m.functions:
        for blk in fn.blocks:
            new_insts = []
            for inst in blk.instructions:
                si = inst.sync_info
                waits = list(si.on_wait) if si and si.on_wait else []
                if len(waits) > max_waits:
                    extra, keep = waits[:-max_waits], waits[-max_waits:]
                    for k, w in enumerate(extra):
                        new_insts.append(
                            mybir.InstNoOp(
                                name=f"{inst.name}_wsplit{k}",
                                engine=inst.engine,
                                ins=[],
                                outs=[],
                                sync_info=mybir.SyncInfo(on_wait=[w], on_update=[]),
                            )
                        )
                    inst.sync_info = mybir.SyncInfo(
                        on_wait=keep,
                        on_update=list(si.on_update) if si.on_update else [],
                    )
                new_insts.append(inst)
            blk.instructions = new_insts


def _build_nc() -> bass.Bass:
    nc = bass.Bass()
    xh = nc.declare_dram_parameter("xh", [I, BS], FP8, isOutput=False)
    xl = nc.declare_dram_parameter("xl", [I, BS], FP8, isOutput=False)
    wh = nc.declare_dram_parameter("wh", [I, OS], FP8, isOutput=False)
    wl = nc.declare_dram_parameter("wl", [I, OS], FP8, isOutput=False)
    b1 = nc.declare_dram_parameter("b1", [P, MT1], F32, isOutput=False)
    w2 = nc.declare_dram_parameter("w2", [OS, O], BF16, isOutput=False)
    # bf16 partials: halves the output write traffic in the serial tail;
    # the host reduces the two j-partials in fp32
    outT = nc.declare_dram_parameter("outT", [O, BS], BF16, isOutput=True)

    with tile.TileContext(nc) as tc:
        with (
            tc.tile_pool(name="const", bufs=1) as const,
            tc.tile_pool(name="xhs", bufs=4) as xhp,
            tc.tile_pool(name="xls", bufs=4) as xlp,
            tc.tile_pool(name="whs", bufs=4) as whp,
            tc.tile_pool(name="wls", bufs=4) as wlp,
            tc.tile_pool(name="fp", bufs=1) as fpool,
            tc.tile_pool(name="op", bufs=3) as opool,
            tc.tile_pool(name="ps1", bufs=1, space="PSUM") as ps1,
            tc.tile_pool(name="ps2", bufs=4, space="PSUM") as ps2,
        ):
            # PE warm-up: ~3.5us of dummy matmuls while the first slabs are
            # still in flight, so the HAM clock gate opens (1.2 -> 2.4 GHz)
            # before the real accumulation begins.
            warm = const.tile([P, N], BF16)
            nc.vector.memset(warm[:], 0.0)
            wps = ps2.tile([P, N], F32, tag="p2g")
            for _ in range(8):
                nc.tensor.matmul(wps[:], warm[:, :P], warm[:],
                                 start=True, stop=True)

            # GEMM1 (fp8 DoubleRow, 3 split-precision chains):
            #   ps[m1] += wh[dkt,m1]^T xh[dkt] + wl^T xh + wh^T xl
            # xh/xl slabs issued from SP (sync), wh/wl from ACT (scalar) so
            # neither engine's descriptor generation is the bottleneck.
            ps = ps1.tile([P, MT1, N], F32)  # 4 PSUM banks, one per m1
            kt0 = 0
            for kb, QK in enumerate(SCHED):
                r0 = kt0 * P
                # host stores each slab block p-major ([P, QK, W] C-order),
                # so every SBUF partition line is one QK*W-byte contiguous
                # DMA descriptor instead of QK separate rows
                xh_s = xhp.tile([P, QMAX, N], FP8, tag="xh")
                nc.sync.dma_start(
                    xh_s[:, :QK, :],
                    xh[r0 : r0 + QK * P, :].rearrange("(p q) n -> p q n", p=P),
                )
                wh_s = whp.tile([P, QMAX, OS], FP8, tag="wh")
                nc.scalar.dma_start(
                    wh_s[:, :QK, :],
                    wh[r0 : r0 + QK * P, :].rearrange("(p q) n -> p q n", p=P),
                )
                xl_s = xlp.tile([P, QMAX, N], FP8, tag="xl")
                nc.sync.dma_start(
                    xl_s[:, :QK, :],
                    xl[r0 : r0 + QK * P, :].rearrange("(p q) n -> p q n", p=P),
                )
                wl_s = wlp.tile([P, QMAX, OS], FP8, tag="wl")
                nc.scalar.dma_start(
                    wl_s[:, :QK, :],
                    wl[r0 : r0 + QK * P, :].rearrange("(p q) n -> p q n", p=P),
                )
                for q2 in range(QK // 2):
                    sl = slice(2 * q2, 2 * q2 + 2)
                    dkt = kt0 // 2 + q2
                    for m1 in range(MT1):
                        msl = slice(m1 * P, (m1 + 1) * P)
                        nc.tensor.matmul(
                            ps[:, m1, :], wh_s[:, sl, msl], xh_s[:, sl, :],
                            start=(dkt == 0), stop=False, perf_mode=DR,
                        )
                        nc.tensor.matmul(
                            ps[:, m1, :], wl_s[:, sl, msl], xh_s[:, sl, :],
                            start=False, stop=False, perf_mode=DR,
                        )
                        nc.tensor.matmul(
                            ps[:, m1, :], wh_s[:, sl, msl], xl_s[:, sl, :],
                            start=False, stop=(dkt == DKT - 1), perf_mode=DR,
                        )
                kt0 += QK

            # constants for the second GEMM (SP has slack between slab
            # triggers; avoiding gpsimd skips its costly SWDGE drain)
            b1_t = const.tile([P, MT1], F32)
            nc.sync.dma_start(b1_t[:], b1[:])
            w2_sb = const.tile([P, KT2, O], BF16)
            for kt in range(KT2):
                nc.sync.dma_start(w2_sb[:, kt, :], w2[kt * P : (kt + 1) * P, :])

            # bias + relu, cast to bf16.  Split DVE/ACT so the last two
            # banks' relus run concurrently and GEMM2's kt=2,3 matmuls
            # aren't serialized behind a single engine.
            f_sb = fpool.tile([P, KT2, N], BF16)
            for m1 in range(MT1):
                if m1 < 2:
                    nc.vector.tensor_scalar(
                        f_sb[:, m1, :],
                        ps[:, m1, :],
                        b1_t[:, m1 : m1 + 1],
                        0.0,
                        mybir.AluOpType.add,
                        mybir.AluOpType.max,
                    )
                else:
                    nc.scalar.activation(
                        f_sb[:, m1, :],
                        ps[:, m1, :],
                        mybir.ActivationFunctionType.Relu,
                        bias=b1_t[:, m1 : m1 + 1],
                    )

            # GEMM2 (partial over this core's hidden half, bf16):
            # outT[m2blk, :] = sum_kt2 w2[kt2blk, m2blk]^T @ fT[kt2blk, :]
            # one m2 per PSUM bank, 4 banks in flight; PSUM->SBUF copies
            # alternate DVE/ACT so the tail copy isn't engine-serialized.
            for m2 in range(MT2):
                p2 = ps2.tile([P, N], F32, tag="p2g")
                for kt in range(KT2):
                    nc.tensor.matmul(
                        p2[:],
                        w2_sb[:, kt, m2 * P : (m2 + 1) * P],
                        f_sb[:, kt, :],
                        start=(kt == 0),
                        stop=(kt == KT2 - 1),
                    )
                ot = opool.tile([P, N], BF16)
                if m2 % 2 == 0:
                    nc.vector.tensor_copy(ot[:], p2[:])
                else:
                    nc.scalar.activation(
                        ot[:], p2[:], mybir.ActivationFunctionType.Copy
                    )
                nc.sync.dma_start(outT[m2 * P : (m2 + 1) * P, :], ot[:])

    _split_multi_waits(nc)
    return nc


def kernel(input, S, THETA, bias, weight, bias2):
    global LAST_RESULTS
    if "nc" not in _CACHE:
        _CACHE["nc"] = _build_nc()
    nc = _CACHE["nc"]

    fp8 = ml_dtypes.float8_e4m3
    f32 = np.float32
    bf16 = ml_dtypes.bfloat16
    input = np.asarray(input, dtype=f32)
    W1 = np.asarray(S, dtype=f32) * np.asarray(THETA, dtype=f32)
    bias = np.asarray(bias, dtype=f32)
    weight = np.asarray(weight, dtype=f32)
    bias2 = np.asarray(bias2, dtype=f32)

    def split8(aT):
        hi = aT.astype(fp8)
        lo = (aT - hi.astype(f32)).astype(fp8)
        return _blockize(hi), _blockize(lo)

    xh_g, xl_g = zip(*(
        split8(np.ascontiguousarray(input[i * BS : (i + 1) * BS, :].T))
        for i in range(R)
    ))
    wh_g, wl_g = zip(*(
        split8(np.ascontiguousarray(W1[j * OS : (j + 1) * OS, :].T) * SC)
        for j in range(C)
    ))
    b1_g = [
        np.ascontiguousarray(bias[j * OS : (j + 1) * OS].reshape(MT1, P).T) * SC
        for j in range(C)
    ]
    w2_g = [(weight[j * OS : (j + 1) * OS, :] / SC).astype(bf16) for j in range(C)]

    in_maps = []
    for i in range(R):
        for j in range(C):
            in_maps.append(
                {"xh": xh_g[i], "xl": xl_g[i], "wh": wh_g[j], "wl": wl_g[j],
                 "b1": b1_g[j], "w2": w2_g[j]}
            )

    res = run_bass_kernel_spmd(
        nc,
        in_maps,
        core_ids=list(range(R * C)),
        trace=checkenv("BASS_TRACE"),
    )
    LAST_RESULTS = res

    out = np.empty((B, O), dtype=np.float32)
    for i in range(R):
        acc = res.results[i * C]["outT"].astype(np.float32)
        for j in range(1, C):
            acc = acc + res.results[i * C + j]["outT"]
        out[i * BS : (i + 1) * BS, :] = acc.T
    out += bias2[None, :]
    return out


# revision 10
# speedup vs baseline: 1.2830x; 1.2830x over previous
"""Trainium2 Bass kernel for nn_CustomModel_52338471469275 (dense MLP).

Computes out = relu(input @ (S*THETA)^T + bias) @ weight + bias2
  input  [2048, 8192] f32
  S,THETA[1024, 8192] f32   (fused on host into W1 = S*THETA)
  weight [1024, 1024] f32
  out    [2048, 1024] f32

Sharding over 8 NeuronCores: 4 batch groups (512 rows each) x 2 hidden
halves (512 of the 1024 hidden units each).  Core (i, j) computes

  fT_ij  = relu(W1[jblk] @ x[iblk]^T + bias[jblk])          # [512, 512]
  outT_p = weight[jblk]^T @ fT_ij                           # [1024, 512]

i.e. a partial (contraction-split) second GEMM.  The host sums the two
j-partials per batch group, transposes, and adds bias2.  No on-device
collectives needed.

All matmul operands are cast to bf16 on the host (fp32 PSUM accumulation
on device).  Measured end-to-end relative error vs the fp32 reference is
~3e-3 (absmax-relative), from bf16 operand rounding.
"""

import os
import sys

import numpy as np

if "/opt/trn_rl_repo" not in sys.path:
    sys.path.insert(0, "/opt/trn_rl_repo")

import ml_dtypes

import concourse.bass as bass
import concourse.tile as tile
from concourse import mybir
from concourse._compat import checkenv
from concourse.bass_utils import run_bass_kernel_spmd

# The image's antenv stub lacks axon_hooks; if BASS_TRACE is set in the
# environment, run_bass_kernel_spmd imports it unconditionally. Provide a
# no-op fallback (trace is skipped, compile+run still work) unless a real
# hook module is already installed.
try:
    import antenv.axon_hooks  # noqa: F401
except ImportError:
    import types

    import antenv

    _hooks = types.ModuleType("antenv.axon_hooks")
    _hooks._hook = None
    _hooks.set_axon_ntff_profile_hook = lambda h: setattr(_hooks, "_hook", h)
    _hooks.get_axon_ntff_profile_hook = lambda: _hooks._hook
    sys.modules["antenv.axon_hooks"] = _hooks
    antenv.axon_hooks = _hooks

B, O, I = 2048, 1024, 8192
R, C = 4, 2                 # batch groups x hidden halves
BS, OS = B // R, O // C     # 512, 512
P = 128
N = BS                      # moving free dim per matmul
KT1 = I // P                # 64 k-tiles, GEMM1
MT1 = OS // P               # 4 m-tiles, GEMM1
KT2 = OS // P               # 4 k-tiles, GEMM2
MT2 = O // P                # 8 m-tiles, GEMM2

BF16 = mybir.dt.bfloat16
F32 = mybir.dt.float32

# k-tiles per slab DMA for GEMM1 (small blocks at the start so the PE gets
# data early, and at the end so the final matmuls aren't gated on a 512 KB
# transfer)
SCHED = [1, 1, 2] + [4] * 14 + [2, 1, 1]
assert sum(SCHED) == KT1


def _blockize(aT):
    """Rewrite [8192, W] so each SCHED block of QK k-tiles is stored p-major
    ([P, QK, W] C-order): one contiguous QK*W-element descriptor per SBUF
    partition instead of QK separate rows."""
    out = np.empty_like(aT)
    kt0 = 0
    for QK in SCHED:
        blk = aT[kt0 * P : (kt0 + QK) * P]
        out[kt0 * P : (kt0 + QK) * P] = (
            blk.reshape(QK, P, -1).transpose(1, 0, 2).reshape(QK * P, -1)
        )
        kt0 += QK
    return out

_CACHE = {}
LAST_RESULTS = None  # BassKernelResults of the most recent run (for test.py)


def _split_multi_waits(nc, max_waits=1):
    """This container's walrus codegen rejects instructions carrying more
    than one semaphore wait ("Too many sync wait commands", CoreV3GenImpl).
    Tile's kernel-tail drain aggregates several; hoist the extras onto
    preceding same-engine NoOps (identical semantics: engines execute their
    stream in order)."""
    for fn in nc.m.functions:
        for blk in fn.blocks:
            new_insts = []
            for inst in blk.instructions:
                si = inst.sync_info
                waits = list(si.on_wait) if si and si.on_wait else []
                if len(waits) > max_waits:
                    extra, keep = waits[:-max_waits], waits[-max_waits:]
                    for k, w in enumerate(extra):
                        new_insts.append(
                            mybir.InstNoOp(
                                name=f"{inst.name}_wsplit{k}",
                                engine=inst.engine,
                                ins=[],
                                outs=[],
                                sync_info=mybir.SyncInfo(on_wait=[w], on_update=[]),
                            )
                        )
                    inst.sync_info = mybir.SyncInfo(
                        on_wait=keep,
                        on_update=list(si.on_update) if si.on_update else [],
                    )
                new_insts.append(inst)
            blk.instructions = new_insts


def _build_nc() -> bass.Bass:
    nc = bass.Bass()
    xT = nc.declare_dram_parameter("xT", [I, BS], BF16, isOutput=False)
    w1T = nc.declare_dram_parameter("w1T", [I, OS], BF16, isOutput=False)
    b1 = nc.declare_dram_parameter("b1", [P, MT1], F32, isOutput=False)
    w2 = nc.declare_dram_parameter("w2", [OS, O], BF16, isOutput=False)
    # bf16 partials: halves the output write traffic in the serial tail;
    # the host reduces the two j-partials in fp32
    outT = nc.declare_dram_parameter("outT", [O, BS], BF16, isOutput=True)

    with tile.TileContext(nc) as tc:
        with (
            tc.tile_pool(name="const", bufs=1) as const,
            tc.tile_pool(name="xs", bufs=5) as xpool,
            tc.tile_pool(name="ws", bufs=5) as wpool,
            tc.tile_pool(name="fp", bufs=1) as fpool,
            tc.tile_pool(name="op", bufs=3) as opool,
            tc.tile_pool(name="ps1", bufs=1, space="PSUM") as ps1,
            tc.tile_pool(name="ps2", bufs=4, space="PSUM") as ps2,
        ):
            # PE warm-up: dummy matmuls while the first slabs are still in
            # flight, so the HAM clock gate opens (1.2 -> 2.4 GHz) early.
            # memset on gpsimd (idle at stream start; DVE's first op would
            # otherwise gate the PE) and short 256-wide matmuls sized to
            # fill the ~2us window before the first slab lands.
            warm = const.tile([P, 256], BF16)
            nc.gpsimd.memset(warm[:], 0.0)
            wps = ps2.tile([P, N], F32, tag="p2g")
            for _ in range(8):
                nc.tensor.matmul(wps[:, :256], warm[:, :P], warm[:],
                                 start=True, stop=True)

            # GEMM1: logitsT[m1blk, :] += W1T[ktblk, m1blk]^T @ xT[ktblk, :]
            # x slabs issued from SP (sync), w1 slabs from ACT (scalar) so
            # neither engine's descriptor generation is the bottleneck.
            ps = ps1.tile([P, MT1, N], F32)  # 4 PSUM banks, one per m1
            kt0 = 0
            for kb, QK in enumerate(SCHED):
                r0 = kt0 * P
                # host stores each slab block p-major ([P, QK, N] C-order),
                # so every SBUF partition line is one QK*N*2-byte contiguous
                # DMA descriptor instead of QK separate 1 KB rows
                xs = xpool.tile([P, 4, N], BF16, tag="xs")
                nc.sync.dma_start(
                    xs[:, :QK, :],
                    xT[r0 : r0 + QK * P, :].rearrange("(p q) n -> p q n", p=P),
                )
                ws = wpool.tile([P, 4, OS], BF16, tag="ws")
                nc.scalar.dma_start(
                    ws[:, :QK, :],
                    w1T[r0 : r0 + QK * P, :].rearrange("(p q) n -> p q n", p=P),
                )
                for q in range(QK):
                    for m1 in range(MT1):
                        nc.tensor.matmul(
                            ps[:, m1, :],
                            ws[:, q, m1 * P : (m1 + 1) * P],
                            xs[:, q, :],
                            start=(kt0 + q == 0),
                            stop=(kt0 + q == KT1 - 1),
                        )
                kt0 += QK

            # constants for the second GEMM (SP has slack between slab
            # triggers; avoiding gpsimd skips its costly SWDGE drain)
            b1_t = const.tile([P, MT1], F32)
            nc.sync.dma_start(b1_t[:], b1[:])
            w2_sb = const.tile([P, KT2, O], BF16)
            for kt in range(KT2):
                nc.sync.dma_start(w2_sb[:, kt, :], w2[kt * P : (kt + 1) * P, :])

            # bias + relu, cast to bf16.  Split DVE/ACT so the last banks'
            # relus run concurrently and GEMM2's kt=2,3 matmuls aren't
            # serialized behind one engine.
            f_sb = fpool.tile([P, KT2, N], BF16)
            for m1 in range(MT1):
                if m1 < 2:
                    nc.vector.tensor_scalar(
                        f_sb[:, m1, :],
                        ps[:, m1, :],
                        b1_t[:, m1 : m1 + 1],
                        0.0,
                        mybir.AluOpType.add,
                        mybir.AluOpType.max,
                    )
                else:
                    nc.scalar.activation(
                        f_sb[:, m1, :],
                        ps[:, m1, :],
                        mybir.ActivationFunctionType.Relu,
                        bias=b1_t[:, m1 : m1 + 1],
                    )

            # GEMM2 (partial over this core's hidden half):
            # outT[m2blk, :] = sum_kt2 w2[kt2blk, m2blk]^T @ fT[kt2blk, :]
            # one m2 per PSUM bank, 4 banks in flight: the first matmul only
            # needs f_sb[:,0,:] (first relu), and copy/DMA of one bank
            # overlaps matmuls of the next three.
            for m2 in range(MT2):
                p2 = ps2.tile([P, N], F32, tag="p2g")
                for kt in range(KT2):
                    nc.tensor.matmul(
                        p2[:],
                        w2_sb[:, kt, m2 * P : (m2 + 1) * P],
                        f_sb[:, kt, :],
                        start=(kt == 0),
                        stop=(kt == KT2 - 1),
                    )
                ot = opool.tile([P, N], BF16)
                if m2 % 2 == 0:
                    nc.vector.tensor_copy(ot[:], p2[:])
                else:
                    nc.scalar.activation(
                        ot[:], p2[:], mybir.ActivationFunctionType.Copy
                    )
                nc.sync.dma_start(outT[m2 * P : (m2 + 1) * P, :], ot[:])

    _split_multi_waits(nc)
    return nc


def kernel(input, S, THETA, bias, weight, bias2):
    global LAST_RESULTS
    if "nc" not in _CACHE:
        _CACHE["nc"] = _build_nc()
    nc = _CACHE["nc"]

    bf16 = ml_dtypes.bfloat16
    input = np.asarray(input, dtype=np.float32)
    W1 = np.asarray(S, dtype=np.float32) * np.asarray(THETA, dtype=np.float32)
    bias = np.asarray(bias, dtype=np.float32)
    weight = np.asarray(weight, dtype=np.float32)
    bias2 = np.asarray(bias2, dtype=np.float32)

    xT_g = [
        _blockize(np.ascontiguousarray(input[i * BS : (i + 1) * BS, :].T).astype(bf16))
        for i in range(R)
    ]
    w1T_g = [
        _blockize(np.ascontiguousarray(W1[j * OS : (j + 1) * OS, :].T).astype(bf16))
        for j in range(C)
    ]
    b1_g = [
        np.ascontiguousarray(bias[j * OS : (j + 1) * OS].reshape(MT1, P).T)
        for j in range(C)
    ]
    w2_g = [weight[j * OS : (j + 1) * OS, :].astype(bf16) for j in range(C)]

    in_maps = []
    for i in range(R):
        for j in range(C):
            in_maps.append(
                {"xT": xT_g[i], "w1T": w1T_g[j], "b1": b1_g[j], "w2": w2_g[j]}
            )

    res = run_bass_kernel_spmd(
        nc,
        in_maps,
        core_ids=list(range(R * C)),
        trace=checkenv("BASS_TRACE"),
    )
    LAST_RESULTS = res

    out = np.empty((B, O), dtype=np.float32)
    for i in range(R):
        acc = res.results[i * C]["outT"].astype(np.float32)
        for j in range(1, C):
            acc = acc + res.results[i * C + j]["outT"]
        out[i * BS : (i + 1) * BS, :] = acc.T
    out += bias2[None, :]
    return out



# revision 17
# speedup vs baseline: 1.3113x; 1.0221x over previous
"""Trainium2 Bass kernel for nn_CustomModel_52338471469275 (dense MLP).

Computes out = relu(input @ (S*THETA)^T + bias) @ weight + bias2
  input  [2048, 8192] f32
  S,THETA[1024, 8192] f32   (fused on host into W1 = S*THETA)
  weight [1024, 1024] f32
  out    [2048, 1024] f32

Sharding over 8 NeuronCores: 4 batch groups (512 rows each) x 2 hidden
halves (512 of the 1024 hidden units each).  Core (i, j) computes

  fT_ij  = relu(W1[jblk] @ x[iblk]^T + bias[jblk])          # [512, 512]
  outT_p = weight[jblk]^T @ fT_ij                           # [1024, 512]

i.e. a partial (contraction-split) second GEMM.  The host sums the two
j-partials per batch group, transposes, and adds bias2.  No on-device
collectives needed.

All matmul operands are cast to bf16 on the host (fp32 PSUM accumulation
on device).  Measured end-to-end relative error vs the fp32 reference is
~3e-3 (absmax-relative), from bf16 operand rounding.
"""

import os
import sys

import numpy as np

if "/opt/trn_rl_repo" not in sys.path:
    sys.path.insert(0, "/opt/trn_rl_repo")

import ml_dtypes

import concourse.bass as bass
import concourse.tile as tile
from concourse import mybir
from concourse._compat import checkenv
from concourse.bass_utils import run_bass_kernel_spmd

# The image's antenv stub lacks axon_hooks; if BASS_TRACE is set in the
# environment, run_bass_kernel_spmd imports it unconditionally. Provide a
# no-op fallback (trace is skipped, compile+run still work) unless a real
# hook module is already installed.
try:
    import antenv.axon_hooks  # noqa: F401
except ImportError:
    import types

    import antenv

    _hooks = types.ModuleType("antenv.axon_hooks")
    _hooks._hook = None
    _hooks.set_axon_ntff_profile_hook = lambda h: setattr(_hooks, "_hook", h)
    _hooks.get_axon_ntff_profile_hook = lambda: _hooks._hook
    sys.modules["antenv.axon_hooks"] = _hooks
    antenv.axon_hooks = _hooks

B, O, I = 2048, 1024, 8192
R, C = 4, 2                 # batch groups x hidden halves
BS, OS = B // R, O // C     # 512, 512
P = 128
N = BS                      # moving free dim per matmul
KT1 = I // P                # 64 k-tiles, GEMM1
MT1 = OS // P               # 4 m-tiles, GEMM1
KT2 = OS // P               # 4 k-tiles, GEMM2
MT2 = O // P                # 8 m-tiles, GEMM2

BF16 = mybir.dt.bfloat16
F32 = mybir.dt.float32

# k-tiles per slab DMA for GEMM1 (small blocks at the start so the PE gets
# data early, and at the end so the final matmuls aren't gated on a 512 KB
# transfer)
SCHED = [1, 1, 2] + [4] * 14 + [2, 1, 1]
assert sum(SCHED) == KT1


def _blockize(aT):
    """Rewrite [8192, W] so each SCHED block of QK k-tiles is stored p-major
    ([P, QK, W] C-order): one contiguous QK*W-element descriptor per SBUF
    partition instead of QK separate rows."""
    out = np.empty_like(aT)
    kt0 = 0
    for QK in SCHED:
        blk = aT[kt0 * P : (kt0 + QK) * P]
        out[kt0 * P : (kt0 + QK) * P] = (
            blk.reshape(QK, P, -1).transpose(1, 0, 2).reshape(QK * P, -1)
        )
        kt0 += QK
    return out

_CACHE = {}
LAST_RESULTS = None  # BassKernelResults of the most recent run (for test.py)


def _split_multi_waits(nc, max_waits=1):
    """This container's walrus codegen rejects instructions carrying more
    than one semaphore wait ("Too many sync wait commands", CoreV3GenImpl).
    Tile's kernel-tail drain aggregates several; hoist the extras onto
    preceding same-engine NoOps (identical semantics: engines execute their
    stream in order)."""
    for fn in nc.m.functions:
        for blk in fn.blocks:
            new_insts = []
            for inst in blk.instructions:
                si = inst.sync_info
                waits = list(si.on_wait) if si and si.on_wait else []
                if len(waits) > max_waits:
                    extra, keep = waits[:-max_waits], waits[-max_waits:]
                    for k, w in enumerate(extra):
                        new_insts.append(
                            mybir.InstNoOp(
                                name=f"{inst.name}_wsplit{k}",
                                engine=inst.engine,
                                ins=[],
                                outs=[],
                                sync_info=mybir.SyncInfo(on_wait=[w], on_update=[]),
                            )
                        )
                    inst.sync_info = mybir.SyncInfo(
                        on_wait=keep,
                        on_update=list(si.on_update) if si.on_update else [],
                    )
                new_insts.append(inst)
            blk.instructions = new_insts


def _build_nc() -> bass.Bass:
    nc = bass.Bass()
    xT = nc.declare_dram_parameter("xT", [I, BS], BF16, isOutput=False)
    w1T = nc.declare_dram_parameter("w1T", [I, OS], BF16, isOutput=False)
    b1 = nc.declare_dram_parameter("b1", [P, MT1], F32, isOutput=False)
    w2 = nc.declare_dram_parameter("w2", [OS, O], BF16, isOutput=False)
    # bf16 partials: halves the output write traffic in the serial tail;
    # the host reduces the two j-partials in fp32
    outT = nc.declare_dram_parameter("outT", [O, BS], BF16, isOutput=True)

    with tile.TileContext(nc) as tc:
        with (
            tc.tile_pool(name="const", bufs=1) as const,
            tc.tile_pool(name="xs", bufs=6) as xpool,
            tc.tile_pool(name="ws", bufs=6) as wpool,
            tc.tile_pool(name="fp", bufs=1) as fpool,
            tc.tile_pool(name="op", bufs=3) as opool,
            tc.tile_pool(name="ps1", bufs=1, space="PSUM") as ps1,
            tc.tile_pool(name="ps2", bufs=4, space="PSUM") as ps2,
        ):
            # PE warm-up: dummy matmuls while the first slabs are still in
            # flight (first-slab trigger->data latency is ~6us; PE stream
            # preamble ends ~7.9us), so the HAM clock gate (needs ~5us of
            # sustained PE activity for 1.2 -> 2.4 GHz) opens just as the
            # real accumulation begins.  10 matmuls fill the 7.9->12.2us
            # window without delaying the first real matmul.
            warm = const.tile([P, N], BF16)
            nc.vector.memset(warm[:], 0.0)
            awarm = const.tile([P, 1], BF16)
            # pre-load ACT's Relu table so the boundary relus don't pay the
            # ~0.6us first-use table switch
            nc.scalar.activation(
                awarm[:], warm[:, :1], mybir.ActivationFunctionType.Relu
            )
            wps = ps2.tile([P, N], F32, tag="p2g")
            for _ in range(10):
                nc.tensor.matmul(wps[:], warm[:, :P], warm[:],
                                 start=True, stop=True)

            # GEMM1: logitsT[m1blk, :] += W1T[ktblk, m1blk]^T @ xT[ktblk, :]
            # x slabs issued from SP (sync), w1 slabs from ACT (scalar) so
            # neither engine's descriptor generation is the bottleneck.
            ps = ps1.tile([P, MT1, N], F32)  # 4 PSUM banks, one per m1
            kt0 = 0
            for kb, QK in enumerate(SCHED):
                r0 = kt0 * P
                # host stores each slab block p-major ([P, QK, N] C-order),
                # so every SBUF partition line is one QK*N*2-byte contiguous
                # DMA descriptor instead of QK separate 1 KB rows
                xs = xpool.tile([P, 4, N], BF16, tag="xs")
                nc.sync.dma_start(
                    xs[:, :QK, :],
                    xT[r0 : r0 + QK * P, :].rearrange("(p q) n -> p q n", p=P),
                )
                ws = wpool.tile([P, 4, OS], BF16, tag="ws")
                nc.scalar.dma_start(
                    ws[:, :QK, :],
                    w1T[r0 : r0 + QK * P, :].rearrange("(p q) n -> p q n", p=P),
                )
                for q in range(QK):
                    for m1 in range(MT1):
                        nc.tensor.matmul(
                            ps[:, m1, :],
                            ws[:, q, m1 * P : (m1 + 1) * P],
                            xs[:, q, :],
                            start=(kt0 + q == 0),
                            stop=(kt0 + q == KT1 - 1),
                        )
                kt0 += QK

            # constants for the second GEMM (SP has slack between slab
            # triggers; avoiding gpsimd skips its costly SWDGE drain)
            b1_t = const.tile([P, MT1], F32)
            nc.sync.dma_start(b1_t[:], b1[:])
            w2_sb = const.tile([P, KT2, O], BF16)
            for kt in range(KT2):
                nc.sync.dma_start(w2_sb[:, kt, :], w2[kt * P : (kt + 1) * P, :])

            # bias + relu, cast to bf16.  Alternate DVE / ACT (gpsimd cannot
            # read PSUM) so consecutive banks' relus run concurrently and
            # GEMM2's kt=1..3 matmuls aren't serialized behind one engine's
            # 0.75us ops.
            f_sb = fpool.tile([P, KT2, N], BF16)
            for m1 in range(MT1):
                if m1 % 2 == 0:
                    nc.vector.tensor_scalar(
                        f_sb[:, m1, :],
                        ps[:, m1, :],
                        b1_t[:, m1 : m1 + 1],
                        0.0,
                        mybir.AluOpType.add,
                        mybir.AluOpType.max,
                    )
                else:
                    nc.scalar.activation(
                        f_sb[:, m1, :],
                        ps[:, m1, :],
                        mybir.ActivationFunctionType.Relu,
                        bias=b1_t[:, m1 : m1 + 1],
                    )

            # GEMM2 (partial over this core's hidden half):
            # outT[m2blk, :] = sum_kt2 w2[kt2blk, m2blk]^T @ fT[kt2blk, :]
            # one m2 per PSUM bank, 4 banks in flight: the first matmul only
            # needs f_sb[:,0,:] (first relu), and copy/DMA of one bank
            # overlaps matmuls of the next three.
            for m2 in range(MT2):
                p2 = ps2.tile([P, N], F32, tag="p2g")
                # the last block's PSUM->SBUF copy + DMA is the serial tail;
                # halve it by accumulating/draining the final m2 in n-halves
                nh = 2 if m2 == MT2 - 1 else 1
                for h in range(nh):
                    hsl = slice(h * (N // nh), (h + 1) * (N // nh))
                    for kt in range(KT2):
                        nc.tensor.matmul(
                            p2[:, hsl],
                            w2_sb[:, kt, m2 * P : (m2 + 1) * P],
                            f_sb[:, kt, hsl],
                            start=(kt == 0),
                            stop=(kt == KT2 - 1),
                        )
                    ot = opool.tile([P, N], BF16)
                    if (m2 + h) % 2 == 0:
                        nc.vector.tensor_copy(ot[:, hsl], p2[:, hsl])
                    else:
                        nc.scalar.activation(
                            ot[:, hsl],
                            p2[:, hsl],
                            mybir.ActivationFunctionType.Copy,
                        )
                    nc.sync.dma_start(
                        outT[m2 * P : (m2 + 1) * P, hsl], ot[:, hsl]
                    )

    _split_multi_waits(nc)
    return nc


def kernel(input, S, THETA, bias, weight, bias2):
    global LAST_RESULTS
    if "nc" not in _CACHE:
        _CACHE["nc"] = _build_nc()
    nc = _CACHE["nc"]

    bf16 = ml_dtypes.bfloat16
    input = np.asarray(input, dtype=np.float32)
    W1 = np.asarray(S, dtype=np.float32) * np.asarray(THETA, dtype=np.float32)
    bias = np.asarray(bias, dtype=np.float32)
    weight = np.asarray(weight, dtype=np.float32)
    bias2 = np.asarray(bias2, dtype=np.float32)

    xT_g = [
        _blockize(np.ascontiguousarray(input[i * BS : (i + 1) * BS, :].T).astype(bf16))
        for i in range(R)
    ]
    w1T_g = [
        _blockize(np.ascontiguousarray(W1[j * OS : (j + 1) * OS, :].T).astype(bf16))
        for j in range(C)
    ]
    b1_g = [
        np.ascontiguousarray(bias[j * OS : (j + 1) * OS].reshape(MT1, P).T)
        for j in range(C)
    ]
    w2_g = [weight[j * OS : (j + 1) * OS, :].astype(bf16) for j in range(C)]

    in_maps = []
    for i in range(R):
        for j in range(C):
            in_maps.append(
                {"xT": xT_g[i], "w1T": w1T_g[j], "b1": b1_g[j], "w2": w2_g[j]}
            )

    res = run_bass_kernel_spmd(
        nc,
        in_maps,
        core_ids=list(range(R * C)),
        trace=checkenv("BASS_TRACE"),
    )
    LAST_RESULTS = res

    out = np.empty((B, O), dtype=np.float32)
    for i in range(R):
        acc = res.results[i * C]["outT"].astype(np.float32)
        for j in range(1, C):
            acc = acc + res.results[i * C + j]["outT"]
        out[i * BS : (i + 1) * BS, :] = acc.T
    out += bias2[None, :]
    return out

